# revision 61
# baseline (speedup 1.0000x reference)
"""Trainium2 Bass kernel for nn_ASAP_81243601371620 (GNN: GraphConv x5 +
ASAPooling x2 + JK-cat MLP head, 16 graphs x 128 nodes).

Sharding: data-parallel over graphs - 2 graphs per NeuronCore, 8 cores.
All message passing / pooling is intra-graph; no collectives. The host
slices inputs per graph, precomputes integer-structure constants from
edge_index (dense per-graph adjacency, one-hot in-neighbor gather
matrices, degree vectors), runs one SPMD Bass program on 8 cores,
gathers the per-core [2,2] logits and applies the row-wise log-softmax
on the host.

v2 structure notes (vs the first working version):
  * input DMA split into 4 ordered chunks over 2 queues so conv1's
    operands land first (~170 KB) instead of after the full 2 MB pack.
  * host also ships x pre-transposed (packed into the unused partition
    rows of the c0 weight columns), removing the two startup PE
    transposes.
  * the two graphs are batched into single instructions through the
    whole pool path: [n, 2n] leaky/exp/softmax tiles, [n, 2] fitness /
    top-k chains, one batched key transpose; softmax normalization is
    folded into the xnew copy (rows) and a P*rec one-hot (columns), so
    the [n,n] softmax scale ACTIVATE disappears.
  * per-purpose PSUM tags (pc/pt/px/pa) so independent sub-chains do
    not false-serialize on a shared buffer pool.
  * A2 is kept unnormalized (diag 1); the 1/K mean normalization is
    folded into the conv aggregation copy scale.
"""
import sys
import functools
import numpy as np
import ml_dtypes

sys.path.insert(0, "/opt/trn_rl_repo")

G = 16
NPG = 128
IN_CH = 64
HID = 128
K1, K2 = 103, 83
NEG_SLOPE = 0.2
SIG_SAT = 16.635532
NCORES = 8
GPC = 2  # graphs per core
BIG = 1.0e30
HEARTBEAT = True  # pulse PE during DVE/scalar chains to keep HAM warm

BF16 = ml_dtypes.bfloat16

# mega pack column map (bf16, [128, MCOLS]), ordered by first use.
# Block A (cols 0:ACOLS) carries conv1+conv2 needs, DMA'd first.
CX = 0               # x node-major: g0 [64] | g1 [64]
CAN = 128            # anorm g0 [128] | g1 [128]
CXT = 384            # parts 0:64: xT g0 [128] | xT g1 [128]
CC0W = 640           # parts 0:64: c0_wrel.T | c0_wroot.T  [256]
CSC = 896            # pw3 [6] | pax [2] | pwq [2] | cbc [4] | c0bc [1]
                     # | l2t [2] | nd1 [2]  = 19
CID = 915            # identity [128]
CCW1 = 1043          # cw_rel[0].T | cw_root[0].T [256]
CBRA = 1299          # row-0: conv1/2 bias rows c0_brel | cb_rel[0] [256]
ACOLS = 1555         # end of block A
# Block B
CONES = 1555         # ones [128]
CAT = 1683           # AT g0 | AT g1 [256]
CATT = 1939          # ATT g0 | ATT g1 [256]
CBGM = 2195          # bigm g0 | g1 [256]
CND = 2451           # NDEGB [2]: -deg cols g0,g1 (pi order)
CLTP = 2453          # LTP g0 | LTP g1 [256]
CIOTA = 2709         # iota row-broadcast [128]
CLT = 2837           # strict lower triangle [128]
CW2 = 2965           # cw_rel[1..3].T | cw_root[1..3].T interleaved [768]
CL1 = 3733           # l1t[0..4] [640]
CB = 4373            # row-0: lin1_b [128] | lin2_b [2]
CBRW2 = 4503         # row-0: conv3/4 bias rows cb_rel[1] | cb_rel[2] [256]
MCOLS = 4759


# ---------------------------------------------------------------- host prep

def _common_grid(ei):
    """Degree-bucket grid shared by all graphs (one SPMD program): nodes
    sorted by in-degree (incl. self), chunks sized so cn*Dc <= 512 where
    Dc is the across-graph max of the sorted-degree envelope."""
    degs = []
    for g in range(G):
        lo = g * NPG
        m = (ei[0] >= lo) & (ei[0] < lo + NPG)
        A = np.zeros((NPG, NPG), bool)
        A[ei[0][m] - lo, ei[1][m] - lo] = True
        np.fill_diagonal(A, True)
        degs.append(np.sort(A.sum(0)))
    env = np.max(np.stack(degs), axis=0)
    grid = []
    i = 0
    while i < NPG:
        j = i
        while j < NPG and (j + 1 - i) * int(env[i:j + 1].max()) <= 512:
            j += 1
        grid.append((i, j - i, int(env[i:j].max())))
        i = j
    return tuple(grid)


def _graph_consts(ei, g, grid):
    """Structure constants for graph g. Pool0 i-indexed tensors are
    permuted into ascending-in-degree order (pi); j-indexed stay in node
    order. ohpack gathers bucketed in-neighbor lists."""
    lo = g * NPG
    m = (ei[0] >= lo) & (ei[0] < lo + NPG)
    src = ei[0][m] - lo
    dst = ei[1][m] - lo
    A = np.zeros((NPG, NPG), np.float32)
    np.add.at(A, (src, dst), 1.0)
    indeg = np.maximum((A != 0).sum(0), 1).astype(np.float32)
    Anorm = A / indeg[None, :]
    At = A.copy()
    np.fill_diagonal(At, 1.0)
    M = At != 0
    deg = M.sum(0)
    pi = np.argsort(deg, kind="stable")
    ts = sum(cn * dc for _, cn, dc in grid)
    ohpack = np.zeros((NPG, ts), np.float32)
    off = 0
    for c0, cn, dc in grid:
        for c in range(cn):
            i = pi[c0 + c]
            nb = np.nonzero(M[:, i])[0]
            col = off + c * dc
            ohpack[nb, col + np.arange(len(nb))] = 1.0
            if len(nb) < dc:
                ohpack[i, col + len(nb):col + dc] = 1.0
        off += cn * dc
    ltp = (pi[None, :] < pi[:, None]).astype(np.float32)
    return dict(
        anorm=Anorm,
        at=At[pi][:, pi].astype(np.float32),         # both axes in pi order
        att=At.T.copy().astype(np.float32),          # node order
        m01=M.T[pi].astype(np.float32),              # 0/1 mask, rows pi
        negdeg=(-deg[pi].astype(np.float32)).reshape(NPG, 1),
        ltp=ltp,
        ohpack=ohpack,
    )


# ---------------------------------------------------------------- program

@functools.lru_cache(maxsize=4)
def _build(grid, scal):
    """Build + compile the SPMD Bass program. `grid` is the colmax bucket
    grid; `scal` is the tuple of scalar bias values baked as immediates."""
    (attb0, attb1, bq0, bq1, le1b0, le1b1, le3b0, le3b1) = scal
    from concourse import bacc, mybir
    from concourse import tile

    f32 = mybir.dt.float32
    bf16 = mybir.dt.bfloat16
    AF = mybir.ActivationFunctionType
    OP = mybir.AluOpType
    AX = mybir.AxisListType
    TS = sum(cn * dc for _, cn, dc in grid)

    nc = bacc.Bacc("TRN2", target_bir_lowering=False, debug=False)

    mega_d = nc.dram_tensor("mega", [128, MCOLS], bf16, kind="ExternalInput")
    ohp_d = nc.dram_tensor("ohpack", [NPG, GPC * TS], bf16,
                           kind="ExternalInput")
    out_d = nc.dram_tensor("out", [GPC, 2], f32, kind="ExternalOutput")

    with tile.TileContext(nc) as tc:
        with (
            tc.tile_pool(name="consts", bufs=1) as cp,
            tc.tile_pool(name="work", bufs=2) as wp,
            tc.tile_pool(name="psum", bufs=2, space="PSUM") as pp,
        ):
            MEGA = cp.tile([128, MCOLS], bf16, name="mega", tag="mega")
            # HAM warm-up: dense dummy matmuls during the input-DMA wait so
            # the PE clock-gate opens before conv1. The scratch tile is
            # memset on the otherwise-idle DVE (a gpsimd memset would queue
            # behind the DMA-issue instructions and push the burst into
            # conv1's window - measured regression).
            WZ = cp.tile([1, 512], bf16, name="wz", tag="wz")
            nc.vector.memset(WZ[:, :], 0.0)
            pwu = pp.tile([128, 512], f32, name="pa", tag="pa", bufs=1)
            for _ in range(6):
                nc.tensor.matmul(pwu[:], WZ[0:1, 0:128], WZ[0:1, :],
                                 start=True, stop=True)
            # ordered input DMA: conv1/2 block split 3-ways (one chunk per
            # engine queue) so it lands at full HBM bandwidth; bulk follows.
            c4 = [0, 519, 1037, ACOLS]
            for eng, lo, hi in zip((nc.sync, nc.gpsimd, nc.scalar),
                                   c4[:-1], c4[1:]):
                eng.dma_start(MEGA[:, lo:hi], mega_d[:, lo:hi])
            nc.sync.dma_start(MEGA[:, ACOLS:MCOLS], mega_d[:, ACOLS:MCOLS])
            OHPB = cp.tile([NPG, GPC * TS], bf16, name="ohpb", tag="ohpb")
            nc.gpsimd.dma_start(OHPB[:, 0:TS], ohp_d[:, 0:TS])
            nc.gpsimd.dma_start(OHPB[:, TS:2 * TS], ohp_d[:, TS:2 * TS])
            # device-built zero helpers (never DMA'd):
            # KB[p]: zero-padded [2, 2n] key-row blocks; ZRQ: zeros for
            # pool1 mask-less logits path is not needed (per-graph ts).
            KB0 = [cp.tile([1, 2 * NPG], bf16, name=f"kb0{g}", tag=f"kb0{g}")
                   for g in range(2)]
            KB1 = [cp.tile([1, 2 * K1], bf16, name=f"kb1{g}", tag=f"kb1{g}")
                   for g in range(2)]
            for g in range(2):
                nc.gpsimd.memset(KB0[g][:, :], 0.0)
                nc.gpsimd.memset(KB1[g][:, :], 0.0)


            IDENT = MEGA[:, CID:CID + 128]
            ONES = MEGA[:, CONES:CONES + 128]
            IOTA = MEGA[:, CIOTA:CIOTA + 128]
            LT = MEGA[:, CLT:CLT + 128]
            C0WREL = MEGA[0:IN_CH, CC0W:CC0W + 128]
            C0WROOT = MEGA[0:IN_CH, CC0W + 128:CC0W + 256]
            XTB = MEGA[0:IN_CH, CXT:CXT + 256]
            CWREL = [MEGA[:, CCW1:CCW1 + 128]] + \
                [MEGA[:, CW2 + 256 * i:CW2 + 256 * i + 128] for i in range(3)]
            CWROOT = [MEGA[:, CCW1 + 128:CCW1 + 256]] + \
                [MEGA[:, CW2 + 256 * i + 128:CW2 + 256 * i + 256]
                 for i in range(3)]
            L1T = [MEGA[:, CL1 + 128 * i:CL1 + 128 * (i + 1)]
                   for i in range(5)]
            PW3 = [MEGA[:, CSC:CSC + 3], MEGA[:, CSC + 3:CSC + 6]]
            PAX = [MEGA[:, CSC + 6:CSC + 7], MEGA[:, CSC + 7:CSC + 8]]
            PWQ = [MEGA[:, CSC + 8:CSC + 9], MEGA[:, CSC + 9:CSC + 10]]
            CBC = [MEGA[:, CSC + 10 + i:CSC + 11 + i] for i in range(4)]
            C0BC = MEGA[:, CSC + 14:CSC + 15]
            L2T = MEGA[:, CSC + 15:CSC + 17]
            ND1 = MEGA[:, CSC + 17:CSC + 19]     # [-K1, -K1] cols (pool1)
            X0 = MEGA[:, CX:CX + IN_CH]
            X1 = MEGA[:, CX + IN_CH:CX + 2 * IN_CH]
            AN = [MEGA[:, CAN + 128 * g:CAN + 128 * (g + 1)]
                  for g in range(2)]
            AT = [MEGA[:, CAT + 128 * g:CAT + 128 * (g + 1)]
                  for g in range(2)]
            ATT = [MEGA[:, CATT + 128 * g:CATT + 128 * (g + 1)]
                   for g in range(2)]
            M01 = MEGA[0:NPG, CBGM:CBGM + 2 * NPG]
            NDEGB = MEGA[:, CND:CND + 2]
            LTP = [MEGA[:, CLTP + 128 * g:CLTP + 128 * (g + 1)]
                   for g in range(2)]
            L1B = MEGA[0:1, CB:CB + 128]
            L2B = MEGA[0:1, CB + 128:CB + 130]
            BRW = [MEGA[0:1, CBRA:CBRA + 128],
                   MEGA[0:1, CBRA + 128:CBRA + 256],
                   MEGA[0:1, CBRW2:CBRW2 + 128],
                   MEGA[0:1, CBRW2 + 128:CBRW2 + 256]]

            def wtile(tag, shape, dt=bf16):
                return wp.tile(shape, dt, name=tag, tag=tag)

            def hb(dep_tile):
                """HAM keep-alive: a 1x1 matmul gated on a just-produced
                tile, pulsing the PE array during long DVE/scalar chains."""
                if not HEARTBEAT:
                    return
                ps = pp.tile([1, 1], f32, name="hb", tag="pt", bufs=3)
                nc.tensor.matmul(ps[:], ONES[0:1, 0:1],
                                 dep_tile[0:1, 0:1], start=True, stop=True)

            def ptile(shape, dt=f32, tag="px", bufs=2):
                return pp.tile(shape, dt, name=tag, tag=tag, bufs=bufs)

            def vcopy(tag, src_ap, shape, dt=bf16):
                t = wtile(tag, shape, dt)
                nc.vector.tensor_copy(t[:], src_ap)
                return t

            def scopy(tag, src_ap, shape, dt=bf16):
                t = wtile(tag, shape, dt)
                nc.scalar.activation(t[:], src_ap, AF.Copy)
                return t

            def transpose(tag, src_ap, n_in, f_in, copy=vcopy):
                """src [n_in part, f_in free] -> sbuf bf16 tile [f_in, n_in]."""
                ps = pp.tile([f_in, n_in], bf16, name="pst", tag="pt",
                             bufs=3)
                nc.tensor.transpose(ps[:], src_ap, IDENT[0:n_in, 0:n_in])
                return copy(tag, ps[:], [f_in, n_in])

            def conv_b(li, n, h0, h1, hTb, c_in, an0, an1,
                       wrelT, wrootT, bcol, brow, xsf, aggscale=1.0,
                       need_nm=True):
                """Batched GraphConv+relu for both graphs.
                h0/h1 [n, c_in] node-major, hTb [c_in, 2n] feature-major.
                Returns (hn0, hn1, hTb_next [HID, 2n]); writes the per-graph
                node-sum readout into xsf[:, 2*li:2*li+2] (fp32).  The
                node-major outputs are produced directly by extra matmuls
                (weight tile as rhs, bias via a ones-row matmul) instead of
                PE transposes, shortening the conv->conv chain."""
                pa = ptile([c_in, 2 * n], tag="pc", bufs=2)
                nc.tensor.matmul(pa[:, 0:n], h0[0:n, 0:c_in], an0,
                                 start=True, stop=True)
                nc.tensor.matmul(pa[:, n:2 * n], h1[0:n, 0:c_in], an1,
                                 start=True, stop=True)
                aggTb = wtile(f"aggT{li}", [c_in, 2 * n])
                nc.scalar.activation(aggTb[:], pa[:], AF.Copy,
                                     scale=aggscale)
                hns = [None, None]
                if need_nm:
                    for g in range(2):
                        ph = pp.tile([n, HID], f32, name="phn", tag="pt",
                                     bufs=3)
                        nc.tensor.matmul(ph[:],
                                         aggTb[:, g * n:(g + 1) * n],
                                         wrelT[0:c_in, :],
                                         start=True, stop=False)
                        nc.tensor.matmul(ph[:],
                                         hTb[0:c_in, g * n:(g + 1) * n],
                                         wrootT[0:c_in, :],
                                         start=False, stop=False)
                        nc.tensor.matmul(ph[:], ONES[0:1, 0:n], brow,
                                         start=False, stop=True)
                        hn = wtile(f"h{li}_{g}", [n, HID])
                        if g == 0:
                            nc.scalar.activation(hn[:], ph[:], AF.Relu)
                        else:
                            nc.vector.tensor_scalar_max(hn[:], ph[:], 0.0)
                        hns[g] = hn
                phT = ptile([HID, 2 * n], tag="pc", bufs=2)
                nc.tensor.matmul(phT[:], wrelT, aggTb[:, :],
                                 start=True, stop=False)
                nc.tensor.matmul(phT[:], wrootT, hTb[0:c_in, 0:2 * n],
                                 start=False, stop=True)
                hTn = wtile(f"hT{li}", [HID, 2 * n])
                nc.scalar.activation(hTn[:], phT[:], AF.Relu, bias=bcol)
                nc.vector.tensor_reduce(
                    xsf[:, 2 * li:2 * li + 2],
                    hTn[:].rearrange("p (g n) -> p g n", g=2),
                    axis=AX.X, op=OP.add)
                return hns[0], hns[1], hTn

            def masked_colmax(g, h_node, qpreTb, col0):
                """degree-bucketed one-hot gather matmuls (transpose mode,
                bf16) + chunked DVE max-reduce; writes qpreT (pi order)
                into qpreTb[:, col0:col0+NPG]."""
                off = 0
                for c0, cn, dc in grid:
                    pg = pp.tile([HID, cn * dc], bf16, name="pg", tag="pt",
                                 bufs=3)
                    nc.tensor.matmul(pg[:], h_node[:, :],
                                     OHPB[:, g * TS + off:
                                          g * TS + off + cn * dc],
                                     start=True, stop=True,
                                     is_transpose=True)
                    nc.vector.tensor_reduce(
                        qpreTb[:, col0 + c0:col0 + c0 + cn],
                        pg[:].rearrange("p (i d) -> p i d", d=dc),
                        axis=AX.X, op=OP.max)
                    off += cn * dc

            def attention_b(pool, n, hT_full, qpreT_full, qw, ax, attbias,
                            dense_q, bgm):
                """Batched both-graph attention.  Returns (se2 [n,2n] bf16
                un-normalized exp rows, rec2 [n,2] f32 row reciprocal)."""
                tg = f"at{pool}"
                # x-leg row + PE broadcast into the logits PSUM
                pxa = ptile([1, 2 * n], tag="px")
                nc.tensor.matmul(pxa[:], ax, hT_full[:, 0:2 * n],
                                 start=True, stop=True)
                xarow = vcopy(f"xarow{tg}", pxa[:], [1, 2 * n])
                plg = pp.tile([n, 2 * n], f32, name="pa", tag="pa", bufs=1)
                nc.tensor.matmul(plg[:], ONES[0:1, 0:n], xarow[0:1, :],
                                 start=True, stop=True)
                # q-leg per-partition column(s)
                if dense_q:
                    pq = ptile([1, 2], tag="px")
                    for g in range(2):
                        nc.tensor.matmul(pq[:, g:g + 1],
                                         qpreT_full[:, g:g + 1], qw,
                                         start=True, stop=True)
                    q1 = wtile(f"q1{tg}", [1, 2])
                    nc.vector.tensor_scalar_add(q1[:], pq[:], attbias)
                    pqb = ptile([n, 2], tag="px")
                    nc.tensor.matmul(pqb[:], ONES[0:1, 0:n], q1[0:1, :],
                                     start=True, stop=True)
                    qabb = wtile(f"qabb{tg}", [n, 2], f32)
                    nc.vector.tensor_copy(qabb[:], pqb[:])
                else:
                    pq = ptile([n, 2], tag="px")
                    for g in range(2):
                        nc.tensor.matmul(pq[:, g:g + 1],
                                         qpreT_full[:, g * n:(g + 1) * n],
                                         qw, start=True, stop=True)
                    qabb = wtile(f"qabb{tg}", [n, 2], f32)
                    nc.vector.tensor_scalar_add(qabb[:], pq[:], attbias)
                # logits (bf16): per-graph q-leg add off the broadcast PSUM,
                # batched bf16 leaky (DVE 2x); mask applied AFTER exp as a
                # 0/1 multiply (exact for masked entries). Lrelu activation
                # deliberately NOT used: it lives in act-table 1 and each
                # switch costs a 1.28us ACT_TABLE_LOAD on the scalar engine.
                lgq = wtile(f"lgq{tg}", [n, 2 * n])
                for g in range(2):
                    nc.vector.tensor_scalar(lgq[:, g * n:(g + 1) * n],
                                            plg[:, g * n:(g + 1) * n],
                                            qabb[:, g:g + 1], None,
                                            op0=OP.add)
                lg2 = wtile(f"lg2{tg}", [n, 2 * n])
                nc.vector.scalar_tensor_tensor(lg2[:], lgq[:], NEG_SLOPE,
                                               lgq[:], op0=OP.mult,
                                               op1=OP.max)
                hb(lgq)
                hb(lg2)
                se2e = wtile(f"se2e{tg}", [n, 2 * n])
                nc.scalar.activation(se2e[:], lg2[:], AF.Exp)
                hb(se2e)
                if bgm is not None:
                    se2 = wtile(f"se2{tg}", [n, 2 * n])
                    nc.vector.tensor_tensor(se2[:], se2e[:], bgm,
                                            op=OP.mult)
                else:
                    se2 = se2e
                dsum2 = wtile(f"dsum2{tg}", [n, 2], f32)
                nc.vector.tensor_reduce(
                    dsum2[:], se2[:].rearrange("p (g n) -> p g n", g=2),
                    axis=AX.X, op=OP.add)
                rec2 = wtile(f"rec2{tg}", [n, 2], f32)
                nc.vector.reciprocal(rec2[:], dsum2[:])
                return se2, rec2

            def fitness_topk_b(pool, n, k, hs, se2, rec2, mfa_lhsT, ndeg_ap,
                               le1b, le3b, w3, ltp, kb):
                """Batched fitness + rank top-k for both graphs.
                Returns per-graph lists (xnew, P, Pf, Prec)."""
                tg = f"ft{pool}"
                # normalized cluster reps; S from PE transpose of se rows
                Ss, xnews, xnewTs = [], [], []
                for g in range(2):
                    S = transpose(f"S{tg}{g}", se2[:, g * n:(g + 1) * n],
                                  n, n, copy=(scopy if g == 0 else vcopy))
                    Ss.append(S)
                    pxn = ptile([n, HID], tag="px" if g == 0 else "pc")
                    nc.tensor.matmul(pxn[:], S[0:n, 0:n], hs[g][0:n, :],
                                     start=True, stop=True)
                    xnew = wtile(f"xnew{tg}{g}", [n, HID])
                    nc.vector.tensor_scalar_mul(xnew[:], pxn[:],
                                                rec2[:, g:g + 1])
                    xnews.append(xnew)
                    xnewTs.append(transpose(f"xnT{tg}{g}", xnew[:], n, HID,
                                            copy=(scopy if g == 0
                                                  else vcopy)))
                # LEConv z in batched [n,2] chains
                paba = ptile([n, 2], tag="px")
                pabb = ptile([n, 2], tag="px")
                pz2 = ptile([n, 2], tag="px")
                for g in range(2):
                    nc.tensor.matmul(paba[:, g:g + 1], xnewTs[g][:, 0:n],
                                     w3[:, 0:1], start=True, stop=True)
                    nc.tensor.matmul(pabb[:, g:g + 1], xnewTs[g][:, 0:n],
                                     w3[:, 1:2], start=True, stop=True)
                acol2 = wtile(f"acol2{tg}", [n, 2])
                nc.vector.tensor_scalar_add(acol2[:], paba[:], le1b)
                for g in range(2):
                    nc.tensor.matmul(pz2[:, g:g + 1], mfa_lhsT[g],
                                     acol2[:, g:g + 1],
                                     start=True, stop=False)
                    nc.tensor.matmul(pz2[:, g:g + 1], xnewTs[g][:, 0:n],
                                     w3[:, 2:3], start=False, stop=True)
                t2a = wtile(f"t2a{tg}", [n, 2], f32)
                nc.vector.tensor_tensor(t2a[:], pabb[:], ndeg_ap,
                                        op=OP.mult)
                t2 = wtile(f"t2{tg}", [n, 2], f32)
                nc.vector.tensor_tensor(t2[:], t2a[:], pz2[:], op=OP.add)
                key2 = wtile(f"key2{tg}", [n, 2])
                nc.vector.tensor_scalar(key2[:], t2[:], le3b, SIG_SAT,
                                        op0=OP.add, op1=OP.min)
                keyf2 = vcopy(f"keyf2{tg}", key2[:], [n, 2], f32)
                hb(key2)
                enz2 = wtile(f"enz2{tg}", [n, 2], f32)
                nc.scalar.activation(enz2[:], t2[:], AF.Exp, scale=-1.0,
                                     bias=-le3b)
                fit2 = wtile(f"fit2{tg}", [n, 2], f32)
                nc.vector.tensor_scalar_add(fit2[:], enz2[:], 1.0)
                nc.vector.reciprocal(fit2[:], fit2[:])
                # per-graph key row transposes -> zero-padded [1, 2n]
                # blocks, summed by two accumulating broadcast matmuls
                for g in range(2):
                    ptk = pp.tile([1, n], bf16, name="pst", tag="pt",
                                  bufs=3)
                    nc.tensor.transpose(ptk[:], key2[:, g:g + 1],
                                        IDENT[0:n, 0:n])
                    if g == 0:
                        nc.scalar.activation(kb[g][0:1, 0:n], ptk[0:1, :],
                                             AF.Copy)
                    else:
                        nc.vector.tensor_copy(kb[g][0:1, n:2 * n],
                                              ptk[0:1, :])
                pfb = pp.tile([n, 2 * n], f32, name="pa", tag="pa", bufs=1)
                nc.tensor.matmul(pfb[:], ONES[0:1, 0:n], kb[0][0:1, :],
                                 start=True, stop=False)
                nc.tensor.matmul(pfb[:], ONES[0:1, 0:n], kb[1][0:1, :],
                                 start=False, stop=True)
                # bf16 SBUF copy of the broadcast keys (exact: products by
                # 1.0 of bf16 keys) so the rank compares run in fast mode
                pfbs = wtile(f"pfbs{tg}", [n, 2 * n])
                nc.scalar.activation(pfbs[:], pfb[:], AF.Copy)
                # rank = #greater + tie-break: compare matrices land in one
                # [n, 4n] tile, then ONE batched reduce (no accumulator
                # round-trips, which serialize the DVE)
                CC = wtile(f"CC{tg}", [n, 4 * n])
                for g in range(2):
                    nc.vector.tensor_scalar(
                        CC[:, 2 * g * n:(2 * g + 1) * n],
                        pfbs[:, g * n:(g + 1) * n],
                        keyf2[:, g:g + 1], None, op0=OP.is_gt)
                    nc.vector.scalar_tensor_tensor(
                        CC[:, (2 * g + 1) * n:(2 * g + 2) * n],
                        pfbs[:, g * n:(g + 1) * n],
                        keyf2[:, g:g + 1], ltp[g],
                        op0=OP.is_equal, op1=OP.mult)
                rank2 = wtile(f"rank2{tg}", [n, 2], f32)
                nc.vector.tensor_reduce(
                    rank2[:], CC[:].rearrange("p (g m) -> p g m", g=2),
                    axis=AX.X, op=OP.add)
                hb(CC)
                Ps, Pfs, Precs = [], [], []
                for g in range(2):
                    P = wtile(f"P{tg}{g}", [n, k])
                    nc.vector.tensor_scalar(P[:], IOTA[0:n, 0:k],
                                            rank2[:, g:g + 1], None,
                                            op0=OP.is_equal)
                    Pf = wtile(f"Pf{tg}{g}", [n, k])
                    nc.vector.tensor_scalar_mul(Pf[:], P[:],
                                                fit2[:, g:g + 1])
                    Prec = wtile(f"Prec{tg}{g}", [n, k])
                    nc.vector.tensor_scalar_mul(Prec[:], P[:],
                                                rec2[:, g:g + 1])
                    Ps.append(P)
                    Pfs.append(Pf)
                    Precs.append(Prec)
                return xnews, Ps, Pfs, Precs

            def coarsen(g, pool, n, k, se_g, Pf, Prec, xnew, atT_lhsT,
                        need_aT, hTb_out, col0):
                """-> (h_out [k,HID], at2 [k,k] diag-1, at2T or None); also
                writes h_outT into hTb_out[:, col0:col0+k]."""
                tg = f"co{pool}{g}"
                tag = "px" if g == 0 else "pc"
                ph = ptile([k, HID], tag=tag)
                nc.tensor.matmul(ph[:], Pf[0:n, 0:k], xnew[0:n, :],
                                 start=True, stop=True)
                h_out = vcopy(f"hp{tg}", ph[:], [k, HID]) if g else \
                    scopy(f"hp{tg}", ph[:], [k, HID])
                phT = ptile([HID, k], tag=tag)
                nc.tensor.matmul(phT[:], xnew[0:n, :], Pf[0:n, 0:k],
                                 start=True, stop=True)
                nc.vector.tensor_copy(hTb_out[:, col0:col0 + k], phT[:])
                psel = ptile([n, k], tag=tag)
                nc.tensor.matmul(psel[:], se_g, Prec[0:n, 0:k],
                                 start=True, stop=True)
                ssel = (scopy if g == 0 else vcopy)(f"ssel{tg}", psel[:],
                                                    [n, k])
                pt1 = ptile([n, k], tag=tag)
                nc.tensor.matmul(pt1[:], atT_lhsT, ssel[:, :],
                                 start=True, stop=True)
                t1 = (scopy if g == 0 else vcopy)(f"t1{tg}", pt1[:],
                                                  [n, k])
                pa2 = ptile([k, k], tag=tag)
                nc.tensor.matmul(pa2[:], ssel[:, :], t1[:, :],
                                 start=True, stop=True)
                at2 = scopy(f"at2{tg}", pa2[:], [k, k])
                nc.gpsimd.affine_select(at2[:], at2[:], [[-1, k]],
                                        compare_op=OP.not_equal, fill=1.0,
                                        base=0, channel_multiplier=1)
                at2T = None
                if need_aT:
                    pa2T = ptile([k, k], tag=tag)
                    nc.tensor.matmul(pa2T[:], t1[:, :], ssel[:, :],
                                     start=True, stop=True)
                    at2T = scopy(f"at2T{tg}", pa2T[:], [k, k])
                    nc.gpsimd.affine_select(at2T[:], at2T[:], [[-1, k]],
                                            compare_op=OP.not_equal,
                                            fill=1.0, base=0,
                                            channel_multiplier=1)
                return h_out, at2, at2T

            # ================= emission =================
            xsf = wtile("xsf", [HID, 10], f32)

            h1_0, h1_1, h1Tb = conv_b(0, NPG, X0, X1, XTB,
                                      IN_CH, AN[0], AN[1],
                                      C0WREL, C0WROOT, C0BC, BRW[0], xsf)
            h2_0, h2_1, h2Tb = conv_b(1, NPG, h1_0, h1_1, h1Tb, HID,
                                      AN[0], AN[1],
                                      CWREL[0], CWROOT[0], CBC[0], BRW[1],
                                      xsf)

            # ---- pool0
            qpreTb = wtile("qpreTb", [HID, 2 * NPG])
            h2s = [h2_0, h2_1]
            for g in range(2):
                masked_colmax(g, h2s[g], qpreTb, g * NPG)
            se0, rec0 = attention_b(0, NPG, h2Tb[:], qpreTb[:],
                                    PWQ[0], PAX[0], attb0 + bq0, False, M01)
            xnews0, Ps0, Pfs0, Precs0 = fitness_topk_b(
                0, NPG, K1, h2s, se0, rec0, [AT[0], AT[1]], NDEGB,
                le1b0, le3b0, PW3[0],
                [LTP[0][0:NPG, 0:NPG], LTP[1][0:NPG, 0:NPG]], KB0)
            h3s, h3Tb = [None, None], wtile("h3Tb", [HID, 2 * K1])
            at2s, at2Ts = [None, None], [None, None]
            for g in range(2):
                h3s[g], at2s[g], at2Ts[g] = coarsen(
                    g, 0, NPG, K1, se0[:, g * NPG:(g + 1) * NPG],
                    Pfs0[g], Precs0[g], xnews0[g], ATT[g],
                    True, h3Tb, g * K1)

            h4_0, h4_1, h4Tb = conv_b(2, K1, h3s[0], h3s[1], h3Tb, HID,
                                      at2s[0][:, :], at2s[1][:, :],
                                      CWREL[1], CWROOT[1], CBC[1], BRW[2],
                                      xsf, aggscale=1.0 / K1)
            h5_0, h5_1, h5Tb = conv_b(3, K1, h4_0, h4_1, h4Tb, HID,
                                      at2s[0][:, :], at2s[1][:, :],
                                      CWREL[2], CWROOT[2], CBC[2], BRW[3],
                                      xsf, aggscale=1.0 / K1)

            # ---- pool1 (dense mask: plain max as the master query)
            h5s = [h5_0, h5_1]
            qpre1b = wtile("qpre1b", [HID, 2])
            nc.vector.tensor_reduce(qpre1b[:],
                                    h5Tb[:].rearrange("p (g n) -> p g n",
                                                      g=2),
                                    axis=AX.X, op=OP.max)
            se1, rec1 = attention_b(1, K1, h5Tb[:], qpre1b[:],
                                    PWQ[1], PAX[1], attb1 + bq1, True, None)
            xnews1, Ps1, Pfs1, Precs1 = fitness_topk_b(
                1, K1, K2, h5s, se1, rec1,
                [ONES[0:K1, 0:K1], ONES[0:K1, 0:K1]], ND1[0:K1, :],
                le1b1, le3b1, PW3[1],
                [LT[0:K1, 0:K1], LT[0:K1, 0:K1]], KB1)
            h6s, h6Tb = [None, None], wtile("h6Tb", [HID, 2 * K2])
            a3s = [None, None]
            for g in range(2):
                h6s[g], a3s[g], _ = coarsen(
                    g, 1, K1, K2, se1[:, g * K1:(g + 1) * K1],
                    Pfs1[g], Precs1[g], xnews1[g], at2Ts[g][:, :],
                    False, h6Tb, g * K2)

            conv_b(4, K2, h6s[0], h6s[1], h6Tb, HID,
                   a3s[0][:, :], a3s[1][:, :],
                   CWREL[3], CWROOT[3], CBC[3], None, xsf,
                   aggscale=1.0 / K2, need_nm=False)

            # ---- MLP head (both graphs batched); log_softmax on host
            xsb = vcopy("xsb", xsf[:], [HID, 10])
            pz = ptile([HID, 2], tag="pc", bufs=2)
            for t_i in range(5):
                nc.tensor.matmul(pz[:], L1T[t_i],
                                 xsb[:, 2 * t_i:2 * t_i + 2],
                                 start=(t_i == 0), stop=False)
            nc.tensor.matmul(pz[:], L1B, ONES[0:1, 0:2],
                             start=False, stop=True)
            zrelu = wtile("zrelu", [HID, 2])
            nc.vector.tensor_scalar_max(zrelu[:], pz[:], 0.0)
            po = ptile([2, 2], tag="pc", bufs=2)
            nc.tensor.matmul(po[:], zrelu[:, :], L2T,
                             start=True, stop=False)
            nc.tensor.matmul(po[:], ONES[0:1, 0:2], L2B,
                             start=False, stop=True)
            res = vcopy("resfin", po[:], [2, 2], f32)
            nc.sync.dma_start(out_d[:], res[:])

    nc.compile()
    return nc


# ---------------------------------------------------------------- host glue

def _prepare(inputs):
    ei = np.asarray(inputs["edge_index"])
    x = np.asarray(inputs["x"], np.float32)
    grid = _common_grid(ei)

    def arr(k):
        return np.ascontiguousarray(np.asarray(inputs[k], np.float32))

    att_w = arr("p_att_w")          # [2, 256]
    lin_w = arr("p_lin_w")          # [2, 128, 128]
    lin_b = arr("p_lin_b")          # [2, 128]
    a_q = att_w[:, :HID]
    a_x = att_w[:, HID:]
    wq = np.einsum("phc,ph->pc", lin_w.transpose(0, 2, 1), a_q)  # lin_w.T@a_q
    bq = np.einsum("ph,ph->p", lin_b, a_q)
    scal = (float(arr("p_att_b")[0]), float(arr("p_att_b")[1]),
            float(bq[0]), float(bq[1]),
            float(arr("p_le1_b")[0]), float(arr("p_le1_b")[1]),
            float(arr("p_le3_b")[0]), float(arr("p_le3_b")[1]))

    ns = [NPG, NPG, K1, K1, K2]
    lin1 = arr("lin1_w")            # [128, 640]
    lin1T = [(lin1[:, t * HID:(t + 1) * HID].T / ns[t]).astype(np.float32)
             for t in range(5)]

    mega = np.zeros((128, MCOLS), np.float32)
    mega[:, CID:CID + 128] = np.eye(128, dtype=np.float32)
    mega[:IN_CH, CC0W:CC0W + 128] = arr("c0_wrel").T
    mega[:IN_CH, CC0W + 128:CC0W + 256] = arr("c0_wroot").T
    mega[:, CCW1:CCW1 + 128] = arr("cw_rel")[0].T
    mega[:, CCW1 + 128:CCW1 + 256] = arr("cw_root")[0].T
    for p in range(2):
        mega[:, CSC + 3 * p:CSC + 3 * p + 3] = np.stack(
            [arr("p_le1_w")[p], arr("p_le2_w")[p], arr("p_le3_w")[p]], 1)
        mega[:, CSC + 6 + p] = a_x[p]
        mega[:, CSC + 8 + p] = wq[p]
    for i in range(4):
        mega[:, CSC + 10 + i] = arr("cb_rel")[i]
    mega[:, CSC + 14] = arr("c0_brel")
    mega[:, CSC + 15:CSC + 17] = arr("lin2_w").T
    mega[:, CSC + 17:CSC + 19] = -float(K1)
    mega[:, CONES:CONES + 128] = 1.0
    mega[:, CIOTA:CIOTA + 128] = np.arange(128, dtype=np.float32)[None, :]
    mega[:, CLT:CLT + 128] = (np.arange(128)[None, :]
                              < np.arange(128)[:, None]).astype(np.float32)
    for i in range(3):
        mega[:, CW2 + 256 * i:CW2 + 256 * i + 128] = arr("cw_rel")[i + 1].T
        mega[:, CW2 + 256 * i + 128:CW2 + 256 * i + 256] = \
            arr("cw_root")[i + 1].T
    for i in range(5):
        mega[:, CL1 + 128 * i:CL1 + 128 * (i + 1)] = lin1T[i]
    mega[0, CB:CB + 128] = arr("lin1_b")
    mega[0, CB + 128:CB + 130] = arr("lin2_b")
    mega[0, CBRA:CBRA + 128] = arr("c0_brel")
    mega[0, CBRA + 128:CBRA + 256] = arr("cb_rel")[0]
    mega[0, CBRW2:CBRW2 + 128] = arr("cb_rel")[1]
    mega[0, CBRW2 + 128:CBRW2 + 256] = arr("cb_rel")[2]

    in_maps = []
    for core in range(NCORES):
        gc = [_graph_consts(ei, core * GPC + j, grid) for j in range(GPC)]
        m = mega.copy()
        for j in range(GPC):
            xg = x[(core * GPC + j) * NPG:(core * GPC + j + 1) * NPG]
            m[:, CX + IN_CH * j:CX + IN_CH * (j + 1)] = xg
            m[0:IN_CH, CXT + 128 * j:CXT + 128 * (j + 1)] = xg.T
            c = gc[j]
            m[:, CAN + 128 * j:CAN + 128 * (j + 1)] = c["anorm"]
            m[:, CAT + 128 * j:CAT + 128 * (j + 1)] = c["at"]
            m[:, CATT + 128 * j:CATT + 128 * (j + 1)] = c["att"]
            m[:, CBGM + 128 * j:CBGM + 128 * (j + 1)] = c["m01"]
            m[:, CND + j] = c["negdeg"][:, 0]
            m[:, CLTP + 128 * j:CLTP + 128 * (j + 1)] = c["ltp"]
        ohp = np.concatenate([gc[j]["ohpack"] for j in range(GPC)], axis=1)
        in_maps.append(dict(mega=m.astype(BF16),
                            ohpack=ohp.astype(BF16)))
    return grid, scal, in_maps


def _log_softmax(z):
    zm = z - z.max(axis=-1, keepdims=True)
    return (zm - np.log(np.exp(zm).sum(axis=-1, keepdims=True))).astype(
        np.float32)


def _run(nc, in_maps, trace=False):
    from concourse.bass_utils import run_bass_kernel_spmd
    return run_bass_kernel_spmd(nc, in_maps, list(range(NCORES)), trace=trace)


def kernel(**inputs):
    grid, scal, in_maps = _prepare(inputs)
    nc = _build(grid, scal)
    res = _run(nc, in_maps)
    z = np.concatenate([res.results[c]["out"] for c in range(NCORES)], 0)
    return _log_softmax(z)


def kernel_traced(**inputs):
    """test.py helper: returns (output, BassKernelResults-with-trace)."""
    grid, scal, in_maps = _prepare(inputs)
    nc = _build(grid, scal)
    res = _run(nc, in_maps, trace=True)
    z = np.concatenate([res.results[c]["out"] for c in range(NCORES)], 0)
    return _log_softmax(z), res


# revision 64
# speedup vs baseline: 1.0093x; 1.0093x over previous
"""Trainium2 Bass kernel for nn_ASAP_81243601371620 (GNN: GraphConv x5 +
ASAPooling x2 + JK-cat MLP head, 16 graphs x 128 nodes).

Sharding: data-parallel over graphs - 2 graphs per NeuronCore, 8 cores.
All message passing / pooling is intra-graph; no collectives. The host
slices inputs per graph, precomputes integer-structure constants from
edge_index (dense per-graph adjacency, one-hot in-neighbor gather
matrices, degree vectors), runs one SPMD Bass program on 8 cores,
gathers the per-core [2,2] logits and applies the row-wise log-softmax
on the host.

v2 structure notes (vs the first working version):
  * input DMA split into 4 ordered chunks over 2 queues so conv1's
    operands land first (~170 KB) instead of after the full 2 MB pack.
  * host also ships x pre-transposed (packed into the unused partition
    rows of the c0 weight columns), removing the two startup PE
    transposes.
  * the two graphs are batched into single instructions through the
    whole pool path: [n, 2n] leaky/exp/softmax tiles, [n, 2] fitness /
    top-k chains, one batched key transpose; softmax normalization is
    folded into the xnew copy (rows) and a P*rec one-hot (columns), so
    the [n,n] softmax scale ACTIVATE disappears.
  * per-purpose PSUM tags (pc/pt/px/pa) so independent sub-chains do
    not false-serialize on a shared buffer pool.
  * A2 is kept unnormalized (diag 1); the 1/K mean normalization is
    folded into the conv aggregation copy scale.
"""
import sys
import functools
import numpy as np
import ml_dtypes

sys.path.insert(0, "/opt/trn_rl_repo")

G = 16
NPG = 128
IN_CH = 64
HID = 128
K1, K2 = 103, 83
NEG_SLOPE = 0.2
SIG_SAT = 16.635532
NCORES = 8
GPC = 2  # graphs per core
BIG = 1.0e30
HEARTBEAT = True  # pulse PE during DVE/scalar chains to keep HAM warm

BF16 = ml_dtypes.bfloat16

# mega pack column map (bf16, [128, MCOLS]), ordered by first use.
# Block A (cols 0:ACOLS) carries conv1+conv2 needs, DMA'd first.
CX = 0               # x node-major: g0 [64] | g1 [64]
CAN = 128            # anorm g0 [128] | g1 [128]
CXT = 384            # parts 0:64: xT g0 [128] | xT g1 [128]
CC0W = 640           # parts 0:64: c0_wrel.T | c0_wroot.T  [256]
CSC = 896            # pw3 [6] | pax [2] | pwq [2] | cbc [4] | c0bc [1]
                     # | l2t [2] | nd1 [2]  = 19
CID = 915            # identity [128]
CCW1 = 1043          # cw_rel[0].T | cw_root[0].T [256]
CBRA = 1299          # row-0: conv1/2 bias rows c0_brel | cb_rel[0] [256]
ACOLS = 1555         # end of block A
# Block B
CONES = 1555         # ones [128]
CAT = 1683           # AT g0 | AT g1 [256]
CATT = 1939          # ATT g0 | ATT g1 [256]
CBGM = 2195          # bigm g0 | g1 [256]
CND = 2451           # NDEGB [2]: -deg cols g0,g1 (pi order)
CLTP = 2453          # LTP g0 | LTP g1 [256]
CIOTA = 2709         # iota row-broadcast [128]
CLT = 2837           # strict lower triangle [128]
CW2 = 2965           # cw_rel[1..3].T | cw_root[1..3].T interleaved [768]
CL1 = 3733           # l1t[0..4] [640]
CB = 4373            # row-0: lin1_b [128] | lin2_b [2]
CBRW2 = 4503         # row-0: conv3/4 bias rows cb_rel[1] | cb_rel[2] [256]
MCOLS = 4759


# ---------------------------------------------------------------- host prep

def _common_grid(ei):
    """Degree-bucket grid shared by all graphs (one SPMD program): nodes
    sorted by in-degree (incl. self), chunks sized so cn*Dc <= 512 where
    Dc is the across-graph max of the sorted-degree envelope."""
    degs = []
    for g in range(G):
        lo = g * NPG
        m = (ei[0] >= lo) & (ei[0] < lo + NPG)
        A = np.zeros((NPG, NPG), bool)
        A[ei[0][m] - lo, ei[1][m] - lo] = True
        np.fill_diagonal(A, True)
        degs.append(np.sort(A.sum(0)))
    env = np.max(np.stack(degs), axis=0)
    grid = []
    i = 0
    while i < NPG:
        j = i
        while j < NPG and (j + 1 - i) * int(env[i:j + 1].max()) <= 512:
            j += 1
        grid.append((i, j - i, int(env[i:j].max())))
        i = j
    return tuple(grid)


def _graph_consts(ei, g, grid):
    """Structure constants for graph g. Pool0 i-indexed tensors are
    permuted into ascending-in-degree order (pi); j-indexed stay in node
    order. ohpack gathers bucketed in-neighbor lists."""
    lo = g * NPG
    m = (ei[0] >= lo) & (ei[0] < lo + NPG)
    src = ei[0][m] - lo
    dst = ei[1][m] - lo
    A = np.zeros((NPG, NPG), np.float32)
    np.add.at(A, (src, dst), 1.0)
    indeg = np.maximum((A != 0).sum(0), 1).astype(np.float32)
    Anorm = A / indeg[None, :]
    At = A.copy()
    np.fill_diagonal(At, 1.0)
    M = At != 0
    deg = M.sum(0)
    pi = np.argsort(deg, kind="stable")
    ts = sum(cn * dc for _, cn, dc in grid)
    ohpack = np.zeros((NPG, ts), np.float32)
    off = 0
    for c0, cn, dc in grid:
        for c in range(cn):
            i = pi[c0 + c]
            nb = np.nonzero(M[:, i])[0]
            col = off + c * dc
            ohpack[nb, col + np.arange(len(nb))] = 1.0
            if len(nb) < dc:
                ohpack[i, col + len(nb):col + dc] = 1.0
        off += cn * dc
    ltp = (pi[None, :] < pi[:, None]).astype(np.float32)
    return dict(
        anorm=Anorm,
        at=At[pi][:, pi].astype(np.float32),         # both axes in pi order
        att=At.T.copy().astype(np.float32),          # node order
        m01=M.T[pi].astype(np.float32),              # 0/1 mask, rows pi
        negdeg=(-deg[pi].astype(np.float32)).reshape(NPG, 1),
        ltp=ltp,
        ohpack=ohpack,
    )


# ---------------------------------------------------------------- program

@functools.lru_cache(maxsize=4)
def _build(grid, scal):
    """Build + compile the SPMD Bass program. `grid` is the colmax bucket
    grid; `scal` is the tuple of scalar bias values baked as immediates."""
    (attb0, attb1, bq0, bq1, le1b0, le1b1, le3b0, le3b1) = scal
    from concourse import bacc, mybir
    from concourse import tile

    f32 = mybir.dt.float32
    bf16 = mybir.dt.bfloat16
    AF = mybir.ActivationFunctionType
    OP = mybir.AluOpType
    AX = mybir.AxisListType
    TS = sum(cn * dc for _, cn, dc in grid)

    nc = bacc.Bacc("TRN2", target_bir_lowering=False, debug=False)

    mega_d = nc.dram_tensor("mega", [128, MCOLS], bf16, kind="ExternalInput")
    ohp_d = nc.dram_tensor("ohpack", [NPG, GPC * TS], bf16,
                           kind="ExternalInput")
    out_d = nc.dram_tensor("out", [GPC, 2], f32, kind="ExternalOutput")

    with tile.TileContext(nc) as tc:
        with (
            tc.tile_pool(name="consts", bufs=1) as cp,
            tc.tile_pool(name="work", bufs=2) as wp,
            tc.tile_pool(name="psum", bufs=2, space="PSUM") as pp,
        ):
            MEGA = cp.tile([128, MCOLS], bf16, name="mega", tag="mega")
            # HAM warm-up: dense dummy matmuls during the input-DMA wait so
            # the PE clock-gate opens before conv1. The scratch tile is
            # memset on the otherwise-idle DVE (a gpsimd memset would queue
            # behind the DMA-issue instructions and push the burst into
            # conv1's window - measured regression).
            WZ = cp.tile([1, 512], bf16, name="wz", tag="wz")
            nc.vector.memset(WZ[:, :], 0.0)
            pwu = pp.tile([128, 512], f32, name="pa", tag="pa", bufs=1)
            for _ in range(8):
                nc.tensor.matmul(pwu[:], WZ[0:1, 0:128], WZ[0:1, :],
                                 start=True, stop=True)
            # ordered input DMA: conv1/2 block split 3-ways (one chunk per
            # engine queue) so it lands at full HBM bandwidth; bulk follows.
            c4 = [0, 519, 1037, ACOLS]
            for eng, lo, hi in zip((nc.sync, nc.gpsimd, nc.scalar),
                                   c4[:-1], c4[1:]):
                eng.dma_start(MEGA[:, lo:hi], mega_d[:, lo:hi])
            nc.sync.dma_start(MEGA[:, ACOLS:MCOLS], mega_d[:, ACOLS:MCOLS])
            OHPB = cp.tile([NPG, GPC * TS], bf16, name="ohpb", tag="ohpb")
            nc.gpsimd.dma_start(OHPB[:, 0:TS], ohp_d[:, 0:TS])
            nc.gpsimd.dma_start(OHPB[:, TS:2 * TS], ohp_d[:, TS:2 * TS])
            # device-built zero helpers (never DMA'd):
            # KB[p]: zero-padded [2, 2n] key-row blocks; ZRQ: zeros for
            # pool1 mask-less logits path is not needed (per-graph ts).
            KB0 = [cp.tile([1, 2 * NPG], bf16, name=f"kb0{g}", tag=f"kb0{g}")
                   for g in range(2)]
            KB1 = [cp.tile([1, 2 * K1], bf16, name=f"kb1{g}", tag=f"kb1{g}")
                   for g in range(2)]
            for g in range(2):
                nc.gpsimd.memset(KB0[g][:, :], 0.0)
                nc.gpsimd.memset(KB1[g][:, :], 0.0)


            IDENT = MEGA[:, CID:CID + 128]
            ONES = MEGA[:, CONES:CONES + 128]
            IOTA = MEGA[:, CIOTA:CIOTA + 128]
            LT = MEGA[:, CLT:CLT + 128]
            C0WREL = MEGA[0:IN_CH, CC0W:CC0W + 128]
            C0WROOT = MEGA[0:IN_CH, CC0W + 128:CC0W + 256]
            XTB = MEGA[0:IN_CH, CXT:CXT + 256]
            CWREL = [MEGA[:, CCW1:CCW1 + 128]] + \
                [MEGA[:, CW2 + 256 * i:CW2 + 256 * i + 128] for i in range(3)]
            CWROOT = [MEGA[:, CCW1 + 128:CCW1 + 256]] + \
                [MEGA[:, CW2 + 256 * i + 128:CW2 + 256 * i + 256]
                 for i in range(3)]
            L1T = [MEGA[:, CL1 + 128 * i:CL1 + 128 * (i + 1)]
                   for i in range(5)]
            PW3 = [MEGA[:, CSC:CSC + 3], MEGA[:, CSC + 3:CSC + 6]]
            PAX = [MEGA[:, CSC + 6:CSC + 7], MEGA[:, CSC + 7:CSC + 8]]
            PWQ = [MEGA[:, CSC + 8:CSC + 9], MEGA[:, CSC + 9:CSC + 10]]
            CBC = [MEGA[:, CSC + 10 + i:CSC + 11 + i] for i in range(4)]
            C0BC = MEGA[:, CSC + 14:CSC + 15]
            L2T = MEGA[:, CSC + 15:CSC + 17]
            ND1 = MEGA[:, CSC + 17:CSC + 19]     # [-K1, -K1] cols (pool1)
            X0 = MEGA[:, CX:CX + IN_CH]
            X1 = MEGA[:, CX + IN_CH:CX + 2 * IN_CH]
            AN = [MEGA[:, CAN + 128 * g:CAN + 128 * (g + 1)]
                  for g in range(2)]
            AT = [MEGA[:, CAT + 128 * g:CAT + 128 * (g + 1)]
                  for g in range(2)]
            ATT = [MEGA[:, CATT + 128 * g:CATT + 128 * (g + 1)]
                   for g in range(2)]
            M01 = MEGA[0:NPG, CBGM:CBGM + 2 * NPG]
            NDEGB = MEGA[:, CND:CND + 2]
            LTP = [MEGA[:, CLTP + 128 * g:CLTP + 128 * (g + 1)]
                   for g in range(2)]
            L1B = MEGA[0:1, CB:CB + 128]
            L2B = MEGA[0:1, CB + 128:CB + 130]
            BRW = [MEGA[0:1, CBRA:CBRA + 128],
                   MEGA[0:1, CBRA + 128:CBRA + 256],
                   MEGA[0:1, CBRW2:CBRW2 + 128],
                   MEGA[0:1, CBRW2 + 128:CBRW2 + 256]]

            def wtile(tag, shape, dt=bf16):
                return wp.tile(shape, dt, name=tag, tag=tag)

            def hb(dep_tile):
                """HAM keep-alive: a 1x1 matmul gated on a just-produced
                tile, pulsing the PE array during long DVE/scalar chains."""
                if not HEARTBEAT:
                    return
                ps = pp.tile([1, 1], f32, name="hb", tag="pt", bufs=3)
                nc.tensor.matmul(ps[:], ONES[0:1, 0:1],
                                 dep_tile[0:1, 0:1], start=True, stop=True)

            def ptile(shape, dt=f32, tag="px", bufs=2):
                return pp.tile(shape, dt, name=tag, tag=tag, bufs=bufs)

            def vcopy(tag, src_ap, shape, dt=bf16):
                t = wtile(tag, shape, dt)
                nc.vector.tensor_copy(t[:], src_ap)
                return t

            def scopy(tag, src_ap, shape, dt=bf16):
                t = wtile(tag, shape, dt)
                nc.scalar.activation(t[:], src_ap, AF.Copy)
                return t

            def transpose(tag, src_ap, n_in, f_in, copy=vcopy):
                """src [n_in part, f_in free] -> sbuf bf16 tile [f_in, n_in]."""
                ps = pp.tile([f_in, n_in], bf16, name="pst", tag="pt",
                             bufs=3)
                nc.tensor.transpose(ps[:], src_ap, IDENT[0:n_in, 0:n_in])
                return copy(tag, ps[:], [f_in, n_in])

            def conv_b(li, n, h0, h1, hTb, c_in, an0, an1,
                       wrelT, wrootT, bcol, brow, xsf, aggscale=1.0,
                       need_nm=True):
                """Batched GraphConv+relu for both graphs.
                h0/h1 [n, c_in] node-major, hTb [c_in, 2n] feature-major.
                Returns (hn0, hn1, hTb_next [HID, 2n]); writes the per-graph
                node-sum readout into xsf[:, 2*li:2*li+2] (fp32).  The
                node-major outputs are produced directly by extra matmuls
                (weight tile as rhs, bias via a ones-row matmul) instead of
                PE transposes, shortening the conv->conv chain."""
                pa = ptile([c_in, 2 * n], tag="pc", bufs=2)
                nc.tensor.matmul(pa[:, 0:n], h0[0:n, 0:c_in], an0,
                                 start=True, stop=True)
                nc.tensor.matmul(pa[:, n:2 * n], h1[0:n, 0:c_in], an1,
                                 start=True, stop=True)
                aggTb = wtile(f"aggT{li}", [c_in, 2 * n])
                nc.scalar.activation(aggTb[:], pa[:], AF.Copy,
                                     scale=aggscale)
                hns = [None, None]
                if need_nm:
                    for g in range(2):
                        ph = pp.tile([n, HID], f32, name="phn", tag="pt",
                                     bufs=3)
                        nc.tensor.matmul(ph[:],
                                         aggTb[:, g * n:(g + 1) * n],
                                         wrelT[0:c_in, :],
                                         start=True, stop=False)
                        nc.tensor.matmul(ph[:],
                                         hTb[0:c_in, g * n:(g + 1) * n],
                                         wrootT[0:c_in, :],
                                         start=False, stop=False)
                        nc.tensor.matmul(ph[:], ONES[0:1, 0:n], brow,
                                         start=False, stop=True)
                        hn = wtile(f"h{li}_{g}", [n, HID])
                        nc.scalar.activation(hn[:], ph[:], AF.Relu)
                        hns[g] = hn
                phT = ptile([HID, 2 * n], tag="pc", bufs=2)
                nc.tensor.matmul(phT[:], wrelT, aggTb[:, :],
                                 start=True, stop=False)
                nc.tensor.matmul(phT[:], wrootT, hTb[0:c_in, 0:2 * n],
                                 start=False, stop=True)
                hTn = wtile(f"hT{li}", [HID, 2 * n])
                nc.scalar.activation(hTn[:], phT[:], AF.Relu, bias=bcol)
                if need_nm:
                    # readout via tiny PE matmuls (h^T @ ones) keeps the
                    # DVE (the busiest engine) free; conv5 has no
                    # node-major output so it uses the DVE reduce
                    pxs = pp.tile([HID, 2], f32, name="pxs", tag="pt",
                                  bufs=3)
                    for g in range(2):
                        nc.tensor.matmul(pxs[:, g:g + 1], hns[g][0:n, :],
                                         ONES[0:n, 0:1],
                                         start=True, stop=True)
                    nc.vector.tensor_copy(xsf[:, 2 * li:2 * li + 2],
                                          pxs[:])
                else:
                    nc.vector.tensor_reduce(
                        xsf[:, 2 * li:2 * li + 2],
                        hTn[:].rearrange("p (g n) -> p g n", g=2),
                        axis=AX.X, op=OP.add)
                return hns[0], hns[1], hTn

            def masked_colmax(g, h_node, qpreTb, col0):
                """degree-bucketed one-hot gather matmuls (transpose mode,
                bf16) + chunked DVE max-reduce; writes qpreT (pi order)
                into qpreTb[:, col0:col0+NPG]."""
                off = 0
                for c0, cn, dc in grid:
                    pg = pp.tile([HID, cn * dc], bf16, name="pg", tag="pt",
                                 bufs=3)
                    nc.tensor.matmul(pg[:], h_node[:, :],
                                     OHPB[:, g * TS + off:
                                          g * TS + off + cn * dc],
                                     start=True, stop=True,
                                     is_transpose=True)
                    nc.vector.tensor_reduce(
                        qpreTb[:, col0 + c0:col0 + c0 + cn],
                        pg[:].rearrange("p (i d) -> p i d", d=dc),
                        axis=AX.X, op=OP.max)
                    off += cn * dc

            def attention_b(pool, n, hT_full, qpreT_full, qw, ax, attbias,
                            dense_q, bgm):
                """Batched both-graph attention.  Returns (se2 [n,2n] bf16
                un-normalized exp rows, rec2 [n,2] f32 row reciprocal)."""
                tg = f"at{pool}"
                # x-leg row + PE broadcast into the logits PSUM
                pxa = ptile([1, 2 * n], tag="px")
                nc.tensor.matmul(pxa[:], ax, hT_full[:, 0:2 * n],
                                 start=True, stop=True)
                xarow = vcopy(f"xarow{tg}", pxa[:], [1, 2 * n])
                plg = pp.tile([n, 2 * n], f32, name="pa", tag="pa", bufs=1)
                nc.tensor.matmul(plg[:], ONES[0:1, 0:n], xarow[0:1, :],
                                 start=True, stop=True)
                # q-leg per-partition column(s)
                if dense_q:
                    pq = ptile([1, 2], tag="px")
                    for g in range(2):
                        nc.tensor.matmul(pq[:, g:g + 1],
                                         qpreT_full[:, g:g + 1], qw,
                                         start=True, stop=True)
                    q1 = wtile(f"q1{tg}", [1, 2])
                    nc.vector.tensor_scalar_add(q1[:], pq[:], attbias)
                    pqb = ptile([n, 2], tag="px")
                    nc.tensor.matmul(pqb[:], ONES[0:1, 0:n], q1[0:1, :],
                                     start=True, stop=True)
                    qabb = wtile(f"qabb{tg}", [n, 2], f32)
                    nc.vector.tensor_copy(qabb[:], pqb[:])
                else:
                    pq = ptile([n, 2], tag="px")
                    for g in range(2):
                        nc.tensor.matmul(pq[:, g:g + 1],
                                         qpreT_full[:, g * n:(g + 1) * n],
                                         qw, start=True, stop=True)
                    qabb = wtile(f"qabb{tg}", [n, 2], f32)
                    nc.vector.tensor_scalar_add(qabb[:], pq[:], attbias)
                # logits (bf16): per-graph q-leg add off the broadcast PSUM,
                # batched bf16 leaky (DVE 2x); mask applied AFTER exp as a
                # 0/1 multiply (exact for masked entries). Lrelu activation
                # deliberately NOT used: it lives in act-table 1 and each
                # switch costs a 1.28us ACT_TABLE_LOAD on the scalar engine.
                lgq = wtile(f"lgq{tg}", [n, 2 * n])
                for g in range(2):
                    nc.vector.tensor_scalar(lgq[:, g * n:(g + 1) * n],
                                            plg[:, g * n:(g + 1) * n],
                                            qabb[:, g:g + 1], None,
                                            op0=OP.add)
                lg2 = wtile(f"lg2{tg}", [n, 2 * n])
                nc.vector.scalar_tensor_tensor(lg2[:], lgq[:], NEG_SLOPE,
                                               lgq[:], op0=OP.mult,
                                               op1=OP.max)
                hb(lgq)
                hb(lg2)
                se2e = wtile(f"se2e{tg}", [n, 2 * n])
                nc.scalar.activation(se2e[:], lg2[:], AF.Exp)
                hb(se2e)
                if bgm is not None:
                    se2 = wtile(f"se2{tg}", [n, 2 * n])
                    nc.vector.tensor_tensor(se2[:], se2e[:], bgm,
                                            op=OP.mult)
                else:
                    se2 = se2e
                dsum2 = wtile(f"dsum2{tg}", [n, 2], f32)
                nc.vector.tensor_reduce(
                    dsum2[:], se2[:].rearrange("p (g n) -> p g n", g=2),
                    axis=AX.X, op=OP.add)
                rec2 = wtile(f"rec2{tg}", [n, 2], f32)
                nc.vector.reciprocal(rec2[:], dsum2[:])
                return se2, rec2

            def fitness_topk_b(pool, n, k, hs, se2, rec2, mfa_lhsT, ndeg_ap,
                               le1b, le3b, w3, ltp, kb):
                """Batched fitness + rank top-k for both graphs.
                Returns per-graph lists (xnew, P, Pf, Prec)."""
                tg = f"ft{pool}"
                # normalized cluster reps; S from PE transpose of se rows
                Ss, xnews, xnewTs = [], [], []
                for g in range(2):
                    S = transpose(f"S{tg}{g}", se2[:, g * n:(g + 1) * n],
                                  n, n, copy=(scopy if g == 0 else vcopy))
                    Ss.append(S)
                    pxn = ptile([n, HID], tag="px" if g == 0 else "pc")
                    nc.tensor.matmul(pxn[:], S[0:n, 0:n], hs[g][0:n, :],
                                     start=True, stop=True)
                    xnew = wtile(f"xnew{tg}{g}", [n, HID])
                    nc.vector.tensor_scalar_mul(xnew[:], pxn[:],
                                                rec2[:, g:g + 1])
                    xnews.append(xnew)
                    xnewTs.append(transpose(f"xnT{tg}{g}", xnew[:], n, HID,
                                            copy=(scopy if g == 0
                                                  else vcopy)))
                # LEConv z in batched [n,2] chains
                paba = ptile([n, 2], tag="px")
                pabb = ptile([n, 2], tag="px")
                pz2 = ptile([n, 2], tag="px")
                for g in range(2):
                    nc.tensor.matmul(paba[:, g:g + 1], xnewTs[g][:, 0:n],
                                     w3[:, 0:1], start=True, stop=True)
                    nc.tensor.matmul(pabb[:, g:g + 1], xnewTs[g][:, 0:n],
                                     w3[:, 1:2], start=True, stop=True)
                acol2 = wtile(f"acol2{tg}", [n, 2])
                nc.vector.tensor_scalar_add(acol2[:], paba[:], le1b)
                for g in range(2):
                    nc.tensor.matmul(pz2[:, g:g + 1], mfa_lhsT[g],
                                     acol2[:, g:g + 1],
                                     start=True, stop=False)
                    nc.tensor.matmul(pz2[:, g:g + 1], xnewTs[g][:, 0:n],
                                     w3[:, 2:3], start=False, stop=True)
                t2a = wtile(f"t2a{tg}", [n, 2], f32)
                nc.vector.tensor_tensor(t2a[:], pabb[:], ndeg_ap,
                                        op=OP.mult)
                t2 = wtile(f"t2{tg}", [n, 2], f32)
                nc.vector.tensor_tensor(t2[:], t2a[:], pz2[:], op=OP.add)
                key2 = wtile(f"key2{tg}", [n, 2])
                nc.vector.tensor_scalar(key2[:], t2[:], le3b, SIG_SAT,
                                        op0=OP.add, op1=OP.min)
                keyf2 = vcopy(f"keyf2{tg}", key2[:], [n, 2], f32)
                hb(key2)
                enz2 = wtile(f"enz2{tg}", [n, 2], f32)
                nc.scalar.activation(enz2[:], t2[:], AF.Exp, scale=-1.0,
                                     bias=-le3b)
                fit2 = wtile(f"fit2{tg}", [n, 2], f32)
                nc.vector.tensor_scalar_add(fit2[:], enz2[:], 1.0)
                nc.vector.reciprocal(fit2[:], fit2[:])
                # per-graph key row transposes -> zero-padded [1, 2n]
                # blocks, summed by two accumulating broadcast matmuls
                for g in range(2):
                    ptk = pp.tile([1, n], bf16, name="pst", tag="pt",
                                  bufs=3)
                    nc.tensor.transpose(ptk[:], key2[:, g:g + 1],
                                        IDENT[0:n, 0:n])
                    if g == 0:
                        nc.scalar.activation(kb[g][0:1, 0:n], ptk[0:1, :],
                                             AF.Copy)
                    else:
                        nc.vector.tensor_copy(kb[g][0:1, n:2 * n],
                                              ptk[0:1, :])
                pfb = pp.tile([n, 2 * n], f32, name="pa", tag="pa", bufs=1)
                nc.tensor.matmul(pfb[:], ONES[0:1, 0:n], kb[0][0:1, :],
                                 start=True, stop=False)
                nc.tensor.matmul(pfb[:], ONES[0:1, 0:n], kb[1][0:1, :],
                                 start=False, stop=True)
                # bf16 SBUF copy of the broadcast keys (exact: products by
                # 1.0 of bf16 keys) so the rank compares run in fast mode
                pfbs = wtile(f"pfbs{tg}", [n, 2 * n])
                nc.scalar.activation(pfbs[:], pfb[:], AF.Copy)
                # rank = #greater + tie-break: compare matrices land in one
                # [n, 4n] tile, then ONE batched reduce (no accumulator
                # round-trips, which serialize the DVE)
                CC = wtile(f"CC{tg}", [n, 4 * n])
                for g in range(2):
                    nc.vector.tensor_scalar(
                        CC[:, 2 * g * n:(2 * g + 1) * n],
                        pfbs[:, g * n:(g + 1) * n],
                        keyf2[:, g:g + 1], None, op0=OP.is_gt)
                    nc.vector.scalar_tensor_tensor(
                        CC[:, (2 * g + 1) * n:(2 * g + 2) * n],
                        pfbs[:, g * n:(g + 1) * n],
                        keyf2[:, g:g + 1], ltp[g],
                        op0=OP.is_equal, op1=OP.mult)
                rank2 = wtile(f"rank2{tg}", [n, 2], f32)
                nc.vector.tensor_reduce(
                    rank2[:], CC[:].rearrange("p (g m) -> p g m", g=2),
                    axis=AX.X, op=OP.add)
                hb(CC)
                Ps, Pfs, Precs = [], [], []
                for g in range(2):
                    P = wtile(f"P{tg}{g}", [n, k])
                    nc.vector.tensor_scalar(P[:], IOTA[0:n, 0:k],
                                            rank2[:, g:g + 1], None,
                                            op0=OP.is_equal)
                    Pf = wtile(f"Pf{tg}{g}", [n, k])
                    nc.vector.tensor_scalar_mul(Pf[:], P[:],
                                                fit2[:, g:g + 1])
                    Prec = wtile(f"Prec{tg}{g}", [n, k])
                    nc.vector.tensor_scalar_mul(Prec[:], P[:],
                                                rec2[:, g:g + 1])
                    Ps.append(P)
                    Pfs.append(Pf)
                    Precs.append(Prec)
                return xnews, Ps, Pfs, Precs

            def coarsen(g, pool, n, k, se_g, Pf, Prec, xnew, atT_lhsT,
                        need_aT, hTb_out, col0):
                """-> (h_out [k,HID], at2 [k,k] diag-1, at2T or None); also
                writes h_outT into hTb_out[:, col0:col0+k]."""
                tg = f"co{pool}{g}"
                tag = "px" if g == 0 else "pc"
                ph = ptile([k, HID], tag=tag)
                nc.tensor.matmul(ph[:], Pf[0:n, 0:k], xnew[0:n, :],
                                 start=True, stop=True)
                h_out = vcopy(f"hp{tg}", ph[:], [k, HID]) if g else \
                    scopy(f"hp{tg}", ph[:], [k, HID])
                phT = ptile([HID, k], tag=tag)
                nc.tensor.matmul(phT[:], xnew[0:n, :], Pf[0:n, 0:k],
                                 start=True, stop=True)
                nc.vector.tensor_copy(hTb_out[:, col0:col0 + k], phT[:])
                psel = ptile([n, k], tag=tag)
                nc.tensor.matmul(psel[:], se_g, Prec[0:n, 0:k],
                                 start=True, stop=True)
                ssel = (scopy if g == 0 else vcopy)(f"ssel{tg}", psel[:],
                                                    [n, k])
                pt1 = ptile([n, k], tag=tag)
                nc.tensor.matmul(pt1[:], atT_lhsT, ssel[:, :],
                                 start=True, stop=True)
                t1 = (scopy if g == 0 else vcopy)(f"t1{tg}", pt1[:],
                                                  [n, k])
                pa2 = ptile([k, k], tag=tag)
                nc.tensor.matmul(pa2[:], ssel[:, :], t1[:, :],
                                 start=True, stop=True)
                at2 = scopy(f"at2{tg}", pa2[:], [k, k])
                nc.gpsimd.affine_select(at2[:], at2[:], [[-1, k]],
                                        compare_op=OP.not_equal, fill=1.0,
                                        base=0, channel_multiplier=1)
                at2T = None
                if need_aT:
                    pa2T = ptile([k, k], tag=tag)
                    nc.tensor.matmul(pa2T[:], t1[:, :], ssel[:, :],
                                     start=True, stop=True)
                    at2T = scopy(f"at2T{tg}", pa2T[:], [k, k])
                    nc.gpsimd.affine_select(at2T[:], at2T[:], [[-1, k]],
                                            compare_op=OP.not_equal,
                                            fill=1.0, base=0,
                                            channel_multiplier=1)
                return h_out, at2, at2T

            # ================= emission =================
            xsf = wtile("xsf", [HID, 10], f32)

            h1_0, h1_1, h1Tb = conv_b(0, NPG, X0, X1, XTB,
                                      IN_CH, AN[0], AN[1],
                                      C0WREL, C0WROOT, C0BC, BRW[0], xsf)
            h2_0, h2_1, h2Tb = conv_b(1, NPG, h1_0, h1_1, h1Tb, HID,
                                      AN[0], AN[1],
                                      CWREL[0], CWROOT[0], CBC[0], BRW[1],
                                      xsf)

            # ---- pool0
            qpreTb = wtile("qpreTb", [HID, 2 * NPG])
            h2s = [h2_0, h2_1]
            for g in range(2):
                masked_colmax(g, h2s[g], qpreTb, g * NPG)
            se0, rec0 = attention_b(0, NPG, h2Tb[:], qpreTb[:],
                                    PWQ[0], PAX[0], attb0 + bq0, False, M01)
            xnews0, Ps0, Pfs0, Precs0 = fitness_topk_b(
                0, NPG, K1, h2s, se0, rec0, [AT[0], AT[1]], NDEGB,
                le1b0, le3b0, PW3[0],
                [LTP[0][0:NPG, 0:NPG], LTP[1][0:NPG, 0:NPG]], KB0)
            h3s, h3Tb = [None, None], wtile("h3Tb", [HID, 2 * K1])
            at2s, at2Ts = [None, None], [None, None]
            for g in range(2):
                h3s[g], at2s[g], at2Ts[g] = coarsen(
                    g, 0, NPG, K1, se0[:, g * NPG:(g + 1) * NPG],
                    Pfs0[g], Precs0[g], xnews0[g], ATT[g],
                    True, h3Tb, g * K1)

            h4_0, h4_1, h4Tb = conv_b(2, K1, h3s[0], h3s[1], h3Tb, HID,
                                      at2s[0][:, :], at2s[1][:, :],
                                      CWREL[1], CWROOT[1], CBC[1], BRW[2],
                                      xsf, aggscale=1.0 / K1)
            h5_0, h5_1, h5Tb = conv_b(3, K1, h4_0, h4_1, h4Tb, HID,
                                      at2s[0][:, :], at2s[1][:, :],
                                      CWREL[2], CWROOT[2], CBC[2], BRW[3],
                                      xsf, aggscale=1.0 / K1)

            # ---- pool1 (dense mask: plain max as the master query)
            h5s = [h5_0, h5_1]
            qpre1b = wtile("qpre1b", [HID, 2])
            nc.vector.tensor_reduce(qpre1b[:],
                                    h5Tb[:].rearrange("p (g n) -> p g n",
                                                      g=2),
                                    axis=AX.X, op=OP.max)
            se1, rec1 = attention_b(1, K1, h5Tb[:], qpre1b[:],
                                    PWQ[1], PAX[1], attb1 + bq1, True, None)
            xnews1, Ps1, Pfs1, Precs1 = fitness_topk_b(
                1, K1, K2, h5s, se1, rec1,
                [ONES[0:K1, 0:K1], ONES[0:K1, 0:K1]], ND1[0:K1, :],
                le1b1, le3b1, PW3[1],
                [LT[0:K1, 0:K1], LT[0:K1, 0:K1]], KB1)
            h6s, h6Tb = [None, None], wtile("h6Tb", [HID, 2 * K2])
            a3s = [None, None]
            for g in range(2):
                h6s[g], a3s[g], _ = coarsen(
                    g, 1, K1, K2, se1[:, g * K1:(g + 1) * K1],
                    Pfs1[g], Precs1[g], xnews1[g], at2Ts[g][:, :],
                    False, h6Tb, g * K2)

            conv_b(4, K2, h6s[0], h6s[1], h6Tb, HID,
                   a3s[0][:, :], a3s[1][:, :],
                   CWREL[3], CWROOT[3], CBC[3], None, xsf,
                   aggscale=1.0 / K2, need_nm=False)

            # ---- MLP head (both graphs batched); log_softmax on host
            xsb = vcopy("xsb", xsf[:], [HID, 10])
            pz = ptile([HID, 2], tag="pc", bufs=2)
            for t_i in range(5):
                nc.tensor.matmul(pz[:], L1T[t_i],
                                 xsb[:, 2 * t_i:2 * t_i + 2],
                                 start=(t_i == 0), stop=False)
            nc.tensor.matmul(pz[:], L1B, ONES[0:1, 0:2],
                             start=False, stop=True)
            zrelu = wtile("zrelu", [HID, 2])
            nc.vector.tensor_scalar_max(zrelu[:], pz[:], 0.0)
            po = ptile([2, 2], tag="pc", bufs=2)
            nc.tensor.matmul(po[:], zrelu[:, :], L2T,
                             start=True, stop=False)
            nc.tensor.matmul(po[:], ONES[0:1, 0:2], L2B,
                             start=False, stop=True)
            res = vcopy("resfin", po[:], [2, 2], f32)
            nc.sync.dma_start(out_d[:], res[:])

    nc.compile()
    return nc


# ---------------------------------------------------------------- host glue

def _prepare(inputs):
    ei = np.asarray(inputs["edge_index"])
    x = np.asarray(inputs["x"], np.float32)
    grid = _common_grid(ei)

    def arr(k):
        return np.ascontiguousarray(np.asarray(inputs[k], np.float32))

    att_w = arr("p_att_w")          # [2, 256]
    lin_w = arr("p_lin_w")          # [2, 128, 128]
    lin_b = arr("p_lin_b")          # [2, 128]
    a_q = att_w[:, :HID]
    a_x = att_w[:, HID:]
    wq = np.einsum("phc,ph->pc", lin_w.transpose(0, 2, 1), a_q)  # lin_w.T@a_q
    bq = np.einsum("ph,ph->p", lin_b, a_q)
    scal = (float(arr("p_att_b")[0]), float(arr("p_att_b")[1]),
            float(bq[0]), float(bq[1]),
            float(arr("p_le1_b")[0]), float(arr("p_le1_b")[1]),
            float(arr("p_le3_b")[0]), float(arr("p_le3_b")[1]))

    ns = [NPG, NPG, K1, K1, K2]
    lin1 = arr("lin1_w")            # [128, 640]
    lin1T = [(lin1[:, t * HID:(t + 1) * HID].T / ns[t]).astype(np.float32)
             for t in range(5)]

    mega = np.zeros((128, MCOLS), np.float32)
    mega[:, CID:CID + 128] = np.eye(128, dtype=np.float32)
    mega[:IN_CH, CC0W:CC0W + 128] = arr("c0_wrel").T
    mega[:IN_CH, CC0W + 128:CC0W + 256] = arr("c0_wroot").T
    mega[:, CCW1:CCW1 + 128] = arr("cw_rel")[0].T
    mega[:, CCW1 + 128:CCW1 + 256] = arr("cw_root")[0].T
    for p in range(2):
        mega[:, CSC + 3 * p:CSC + 3 * p + 3] = np.stack(
            [arr("p_le1_w")[p], arr("p_le2_w")[p], arr("p_le3_w")[p]], 1)
        mega[:, CSC + 6 + p] = a_x[p]
        mega[:, CSC + 8 + p] = wq[p]
    for i in range(4):
        mega[:, CSC + 10 + i] = arr("cb_rel")[i]
    mega[:, CSC + 14] = arr("c0_brel")
    mega[:, CSC + 15:CSC + 17] = arr("lin2_w").T
    mega[:, CSC + 17:CSC + 19] = -float(K1)
    mega[:, CONES:CONES + 128] = 1.0
    mega[:, CIOTA:CIOTA + 128] = np.arange(128, dtype=np.float32)[None, :]
    mega[:, CLT:CLT + 128] = (np.arange(128)[None, :]
                              < np.arange(128)[:, None]).astype(np.float32)
    for i in range(3):
        mega[:, CW2 + 256 * i:CW2 + 256 * i + 128] = arr("cw_rel")[i + 1].T
        mega[:, CW2 + 256 * i + 128:CW2 + 256 * i + 256] = \
            arr("cw_root")[i + 1].T
    for i in range(5):
        mega[:, CL1 + 128 * i:CL1 + 128 * (i + 1)] = lin1T[i]
    mega[0, CB:CB + 128] = arr("lin1_b")
    mega[0, CB + 128:CB + 130] = arr("lin2_b")
    mega[0, CBRA:CBRA + 128] = arr("c0_brel")
    mega[0, CBRA + 128:CBRA + 256] = arr("cb_rel")[0]
    mega[0, CBRW2:CBRW2 + 128] = arr("cb_rel")[1]
    mega[0, CBRW2 + 128:CBRW2 + 256] = arr("cb_rel")[2]

    in_maps = []
    for core in range(NCORES):
        gc = [_graph_consts(ei, core * GPC + j, grid) for j in range(GPC)]
        m = mega.copy()
        for j in range(GPC):
            xg = x[(core * GPC + j) * NPG:(core * GPC + j + 1) * NPG]
            m[:, CX + IN_CH * j:CX + IN_CH * (j + 1)] = xg
            m[0:IN_CH, CXT + 128 * j:CXT + 128 * (j + 1)] = xg.T
            c = gc[j]
            m[:, CAN + 128 * j:CAN + 128 * (j + 1)] = c["anorm"]
            m[:, CAT + 128 * j:CAT + 128 * (j + 1)] = c["at"]
            m[:, CATT + 128 * j:CATT + 128 * (j + 1)] = c["att"]
            m[:, CBGM + 128 * j:CBGM + 128 * (j + 1)] = c["m01"]
            m[:, CND + j] = c["negdeg"][:, 0]
            m[:, CLTP + 128 * j:CLTP + 128 * (j + 1)] = c["ltp"]
        ohp = np.concatenate([gc[j]["ohpack"] for j in range(GPC)], axis=1)
        in_maps.append(dict(mega=m.astype(BF16),
                            ohpack=ohp.astype(BF16)))
    return grid, scal, in_maps


def _log_softmax(z):
    zm = z - z.max(axis=-1, keepdims=True)
    return (zm - np.log(np.exp(zm).sum(axis=-1, keepdims=True))).astype(
        np.float32)


def _run(nc, in_maps, trace=False):
    from concourse.bass_utils import run_bass_kernel_spmd
    return run_bass_kernel_spmd(nc, in_maps, list(range(NCORES)), trace=trace)


def kernel(**inputs):
    grid, scal, in_maps = _prepare(inputs)
    nc = _build(grid, scal)
    res = _run(nc, in_maps)
    z = np.concatenate([res.results[c]["out"] for c in range(NCORES)], 0)
    return _log_softmax(z)


def kernel_traced(**inputs):
    """test.py helper: returns (output, BassKernelResults-with-trace)."""
    grid, scal, in_maps = _prepare(inputs)
    nc = _build(grid, scal)
    res = _run(nc, in_maps, trace=True)
    z = np.concatenate([res.results[c]["out"] for c in range(NCORES)], 0)
    return _log_softmax(z), res
